# revision 15
# baseline (speedup 1.0000x reference)
"""Trainium2 Bass kernel for a single-step Elman RNN cell + linear + softmax.

Reference computation (B=256, I=H=O=4096, fp32):
    hn     = tanh(x @ w_ih.T + b_ih + h0[0] @ w_hh.T + b_hh)      # [B, H]
    logits = hn @ w_lin.T + b_lin                                  # [B, O]
    probs  = softmax(logits, axis=-1)
    return probs[None], hn[None]

Sharding (8 cores, tensor-parallel): core c owns rows hs = [512c, 512c+512)
of H (and the same slice of O).
  Phase 1: each core computes hnT_c = tanh(W_ih[hs] @ x.T + W_hh[hs] @ h.T + b)
           as [512, 256] (H on partitions, batch on free dim), in G column
           groups; each group's [256, 256] result is AllGathered while the
           next group (and then phase 2) computes, hiding collective latency.
  Phase 2: each core computes its O-slice of logits: [256, 512] =
           (hnT k-tiles).T @ w_lin[os].T, batch on partitions.
  Softmax: exp on-chip; per-core partial row sums are AllGathered (tiny) and
           summed so every core normalizes its O-slice with the global denom.

All matmul operands are pre-transposed on the host so the contraction dim (I
resp. H) lands on SBUF partitions and every DMA is contiguous.

hnT SBUF layout: the gathered hnT k-tiles are stored in (group, rank, kk)
order, index j = g*(KT/G) + r*(KT/(G*NCORES... )); phase 2 maps global k-tile
k = 4r + 2g + kk  ->  j = g*16 + r*2 + kk (for G=2) so each group's AllGather
lands contiguously while matmuls read the right tile.
"""

import os

import numpy as np

import concourse.bass as bass
import concourse.mybir as mybir
import concourse.tile as tile
from concourse import bacc
from concourse.bass import ts
from concourse.bass_utils import run_bass_kernel_spmd

NCORES = 8
B = 256
I = H = O = 4096
SH = H // NCORES  # 512: per-core shard of H / O
P = 128
KT = I // P  # 32 k-tiles
MS = SH // P  # 4 m-tiles (H-shard)
BT = B // P  # 2 batch tiles
G = 2  # phase-1 column groups (pipelined AllGathers)
MG = MS // G  # m-tiles per group
GW = SH // G  # group width (columns of the shard)

F32 = mybir.dt.float32
BF16 = mybir.dt.bfloat16
FP16 = mybir.dt.float16

# Matmul precision mode: "fp32" (exact, 4 cyc/row), "fp16" (1 cyc/row,
# ~5e-4 rel err), "bf16" (1 cyc/row, ~3e-3 rel err).
MODE = os.environ.get("RNN_MODE", "fp16")

# k-tile groups per weight-slab DMA: first slabs small for a fast start.
P1_SLABS = [1, 3, 4, 8, 8, 8]
P2_SLABS = [2, 3, 3, 8, 8, 8]

_cache: dict = {}


def _mm_dt(mode):
    return {"fp32": F32, "bf16": BF16, "fp16": FP16}[mode]


def _emit(nc, tc, mode):
    mdt = _mm_dt(mode)

    # ---- DRAM I/O ----
    xT = nc.dram_tensor("xT", [I, B], mdt, kind="ExternalInput")
    hT = nc.dram_tensor("hT", [H, B], mdt, kind="ExternalInput")
    wih = nc.dram_tensor("wih", [I, SH], mdt, kind="ExternalInput")
    whh = nc.dram_tensor("whh", [H, SH], mdt, kind="ExternalInput")
    wlin = nc.dram_tensor("wlin", [H, SH], mdt, kind="ExternalInput")
    blin = nc.dram_tensor("blin", [1, SH], mdt, kind="ExternalInput")
    b1 = nc.dram_tensor("b1", [P, MS], F32, kind="ExternalInput")  # (b_ih+b_hh)[hs]

    probs_out = nc.dram_tensor("probs_s", [B, SH], F32, kind="ExternalOutput")
    hn_out = nc.dram_tensor("hn_s", [SH, B], F32, kind="ExternalOutput")

    rg = [list(range(NCORES))]

    with (
        tc.tile_pool(name="const", bufs=1) as const_pool,
        tc.tile_pool(name="acts", bufs=1) as acts_pool,
        tc.tile_pool(name="w1", bufs=4) as w1_pool,
        tc.tile_pool(name="ps1", bufs=1, space="PSUM") as ps1_pool,
        tc.tile_pool(name="ps2", bufs=1, space="PSUM") as ps2_pool,
        tc.tile_pool(name="dram", bufs=1, space="DRAM") as dram_pool,
    ):
        # ---- constants ----
        b1_sb = const_pool.tile([P, MS], F32)
        nc.sync.dma_start(b1_sb[:], b1.ap())
        blin_sb = const_pool.tile([1, SH], mdt)
        nc.sync.dma_start(blin_sb[:], blin.ap())
        ones_sb = const_pool.tile([1, B], mdt)
        nc.vector.memset(ones_sb[:], 1.0)

        # ---- resident activations ----
        xT_sb = acts_pool.tile([P, KT, B], mdt)
        hT_sb = acts_pool.tile([P, KT, B], mdt)
        hnT_sb = acts_pool.tile([P, KT, B], mdt)  # gathered full hnT (j-order)
        hn32_sb = acts_pool.tile([P, MS, B], F32)  # own shard, fp32 (output)
        if mode == "fp32":
            hnmm_sb = hn32_sb
        else:
            hnmm_sb = acts_pool.tile([P, MS, B], mdt)

        # collective bounce buffers (one pair per phase-1 group)
        cc1_in = []
        cc1_out = []
        for g in range(G):
            t_in = dram_pool.tile([GW, B], mdt, name=f"cc1_in_{g}")
            t_out = dram_pool.tile(
                [GW * NCORES, B], mdt, addr_space="Shared", name=f"cc1_out_{g}"
            )
            cc1_in.append(t_in)
            cc1_out.append(t_out)
        cc2_in = dram_pool.tile([B, 1], F32)
        cc2_out = dram_pool.tile([B * NCORES, 1], F32, addr_space="Shared")

        # ---- phase 1, grouped; group g covers shard columns [g*GW,(g+1)*GW) ----
        ps1 = [
            ps1_pool.tile([P, B], F32, tag=f"ps1_{m}", name=f"ps1_{m}")
            for m in range(MS)
        ]

        for g in range(G):
            gsl = slice(g * GW, (g + 1) * GW)
            pos = 0
            for si, nk in enumerate(P1_SLABS):
                ksl = slice(pos * P, (pos + nk) * P)
                if g == 0:
                    nc.sync.dma_start(
                        xT_sb[:, pos : pos + nk, :],
                        xT.ap()[ksl, :].rearrange("(kk p) b -> p kk b", p=P),
                    )
                    nc.sync.dma_start(
                        hT_sb[:, pos : pos + nk, :],
                        hT.ap()[ksl, :].rearrange("(kk p) b -> p kk b", p=P),
                    )
                wih_sb = w1_pool.tile([P, 8, GW], mdt, tag="wih", name=f"wih_{g}_{si}")
                nc.sync.dma_start(
                    wih_sb[:, :nk, :],
                    wih.ap()[ksl, gsl].rearrange("(kk p) s -> p kk s", p=P),
                )
                whh_sb = w1_pool.tile([P, 8, GW], mdt, tag="whh", name=f"whh_{g}_{si}")
                nc.sync.dma_start(
                    whh_sb[:, :nk, :],
                    whh.ap()[ksl, gsl].rearrange("(kk p) s -> p kk s", p=P),
                )
                for kk in range(nk):
                    k = pos + kk
                    for mg in range(MG):
                        m = g * MG + mg
                        nc.tensor.matmul(
                            ps1[m][:],
                            lhsT=wih_sb[:, kk, ts(mg, P)],
                            rhs=xT_sb[:, k, :],
                            start=(k == 0),
                            stop=False,
                        )
                        nc.tensor.matmul(
                            ps1[m][:],
                            lhsT=whh_sb[:, kk, ts(mg, P)],
                            rhs=hT_sb[:, k, :],
                            start=False,
                            stop=(k == KT - 1),
                        )
                pos += nk

            for mg in range(MG):
                m = g * MG + mg
                nc.scalar.activation(
                    hn32_sb[:, m, :],
                    ps1[m][:],
                    mybir.ActivationFunctionType.Tanh,
                    bias=b1_sb[:, m : m + 1],
                )
                if mode != "fp32":
                    nc.scalar.activation(
                        hnmm_sb[:, m, :],
                        ps1[m][:],
                        mybir.ActivationFunctionType.Tanh,
                        bias=b1_sb[:, m : m + 1],
                    )

            # group shard -> DRAM -> AllGather -> hnT_sb[:, g*16:(g+1)*16, :]
            nc.scalar.dma_start(
                cc1_in[g].rearrange("(m p) b -> p m b", p=P),
                hnmm_sb[:, g * MG : (g + 1) * MG, :],
            )
            nc.gpsimd.collective_compute(
                "AllGather",
                mybir.AluOpType.bypass,
                replica_groups=rg,
                ins=[cc1_in[g][:]],
                outs=[cc1_out[g][:]],
            )
            JP = KT // G  # j-tiles per group (16)
            cc1_view = cc1_out[g].rearrange("(rk p) b -> p rk b", p=P)
            HJ = JP // 2
            nc.gpsimd.dma_start(
                hnT_sb[:, g * JP : g * JP + HJ, :], cc1_view[:, :HJ, :]
            )
            nc.scalar.dma_start(
                hnT_sb[:, g * JP + HJ : (g + 1) * JP, :], cc1_view[:, HJ:, :]
            )

        # own hn shard is final now; store it early (overlaps phase 2).
        # On scalar: sync must keep streaming phase-2 weight slabs.
        nc.scalar.dma_start(hn_out.ap().rearrange("(m p) b -> p m b", p=P), hn32_sb[:])

        # ---- phase 2: logits_c = hnT.T @ wlin (+ blin via ones-row) ----
        # hnT_sb is in j-order (group-major), and the host pre-permutes wlin's
        # rows into the same j-order, so iterating j consumes group 0's tiles
        # first (phase 2 starts as soon as AllGather 0 lands). wlin is fully
        # SBUF-resident: its 4 big DMAs queue on sync behind the phase-1
        # slabs and stream during the AllGather window when HBM is otherwise
        # idle. The bias (ones-row) matmul OPENS each accumulation group so
        # the group closes on the last j-matmul and exp can start immediately.
        ps2 = [
            ps2_pool.tile([P, SH], F32, tag=f"ps2_{mb}", name=f"ps2_{mb}")
            for mb in range(BT)
        ]
        wlin_sb = acts_pool.tile([P, KT, SH], mdt)
        for ci in range(4):
            ksl = slice(ci * 8 * P, (ci + 1) * 8 * P)
            nc.sync.dma_start(
                wlin_sb[:, ci * 8 : (ci + 1) * 8, :],
                wlin.ap()[ksl, :].rearrange("(kk p) s -> p kk s", p=P),
            )
        for mb in range(BT):
            nc.tensor.matmul(
                ps2[mb][:],
                lhsT=ones_sb[:, ts(mb, P)],
                rhs=blin_sb[:],
                start=True,
                stop=False,
            )
        for j in range(KT):
            for mb in range(BT):
                nc.tensor.matmul(
                    ps2[mb][:],
                    lhsT=hnT_sb[:, j, ts(mb, P)],
                    rhs=wlin_sb[:, j, :],
                    start=False,
                    stop=(j == KT - 1),
                )

        # ---- softmax over full O (partial sums exchanged via AllGather) ----
        probs_sb = acts_pool.tile([P, BT, SH], F32)
        part_sb = acts_pool.tile([P, BT], F32)
        sums_sb = acts_pool.tile([P, BT, NCORES], F32)
        den_sb = acts_pool.tile([P, BT], F32)
        rden_sb = acts_pool.tile([P, BT], F32)

        for mb in range(BT):
            nc.scalar.activation(
                probs_sb[:, mb, :], ps2[mb][:], mybir.ActivationFunctionType.Exp
            )
            nc.vector.reduce_sum(
                part_sb[:, mb : mb + 1], probs_sb[:, mb, :], axis=mybir.AxisListType.X
            )
        nc.gpsimd.dma_start(cc2_in.rearrange("(m p) o -> p (m o)", p=P), part_sb[:])
        nc.gpsimd.collective_compute(
            "AllGather",
            mybir.AluOpType.bypass,
            replica_groups=rg,
            ins=[cc2_in[:]],
            outs=[cc2_out[:]],
        )
        cc2_view = cc2_out.rearrange("(r m p) o -> m p (r o)", r=NCORES, p=P)
        nc.gpsimd.dma_start(sums_sb[:, 0, :], cc2_view[0])
        nc.scalar.dma_start(sums_sb[:, 1, :], cc2_view[1])
        for mb in range(BT):
            nc.vector.reduce_sum(
                den_sb[:, mb : mb + 1], sums_sb[:, mb, :], axis=mybir.AxisListType.X
            )
        nc.vector.reciprocal(rden_sb[:], den_sb[:])
        probs_view = probs_out.ap().rearrange("(m p) o -> p m o", p=P)
        for mb in range(BT):
            nc.vector.tensor_scalar_mul(
                probs_sb[:, mb, :], probs_sb[:, mb, :], rden_sb[:, mb : mb + 1]
            )
            nc.sync.dma_start(probs_view[:, mb, :], probs_sb[:, mb, :])


def _build(mode):
    if mode in _cache:
        return _cache[mode]
    nc = bacc.Bacc(
        "TRN2",
        target_bir_lowering=False,
        debug=False,
        num_devices=NCORES,
    )
    with tile.TileContext(nc) as tc:
        _emit(nc, tc, mode)
    nc.compile()
    _cache[mode] = nc
    return nc


def _np_dt(mode):
    if mode == "bf16":
        import ml_dtypes

        return ml_dtypes.bfloat16
    if mode == "fp16":
        return np.float16
    return np.float32


def _prep_in_maps(x, h0, w_ih, b_ih, w_hh, b_hh, w_lin, b_lin, mode):
    dt = _np_dt(mode)
    x = np.asarray(x, np.float32)
    h = np.asarray(h0, np.float32).reshape(B, H)
    w_ih = np.asarray(w_ih, np.float32)
    w_hh = np.asarray(w_hh, np.float32)
    w_lin = np.asarray(w_lin, np.float32)
    b1_full = np.asarray(b_ih, np.float32) + np.asarray(b_hh, np.float32)
    b_lin = np.asarray(b_lin, np.float32)

    xT = np.ascontiguousarray(x.T).astype(dt, copy=False)
    hT = np.ascontiguousarray(h.T).astype(dt, copy=False)

    in_maps = []
    for c in range(NCORES):
        hs = slice(c * SH, (c + 1) * SH)
        # wlin rows permuted to match hnT_sb's j-order (group-major): global
        # k-tile k = MS*r + MG*g + kk lands at j = g*(KT//G) + r*MG + kk.
        wlt = np.ascontiguousarray(w_lin[hs].T).astype(dt, copy=False)
        wlt_j = np.ascontiguousarray(
            wlt.reshape(NCORES, G, MG, P, SH)
            .transpose(1, 0, 2, 3, 4)
            .reshape(H, SH)
        )
        in_maps.append(
            {
                "xT": xT,
                "hT": hT,
                "wih": np.ascontiguousarray(w_ih[hs].T).astype(dt, copy=False),
                "whh": np.ascontiguousarray(w_hh[hs].T).astype(dt, copy=False),
                "wlin": wlt_j,
                "blin": np.ascontiguousarray(b_lin[hs][None, :]).astype(dt, copy=False),
                "b1": np.ascontiguousarray(b1_full[hs].reshape(MS, P).T),
            }
        )
    return in_maps


def _gather(results):
    probs = np.concatenate([results[c]["probs_s"] for c in range(NCORES)], axis=1)
    hnT = np.concatenate([results[c]["hn_s"] for c in range(NCORES)], axis=0)
    hn = np.ascontiguousarray(hnT.T)
    return probs[None, :, :], hn[None, :, :]


def run(inputs, mode=None, **spmd_kwargs):
    mode = mode or MODE
    nc = _build(mode)
    in_maps = _prep_in_maps(**inputs, mode=mode)
    res = run_bass_kernel_spmd(nc, in_maps, core_ids=list(range(NCORES)), **spmd_kwargs)
    return _gather(res.results), res


def kernel(x, h0, w_ih, b_ih, w_hh, b_hh, w_lin, b_lin):
    out, _ = run(
        dict(
            x=x, h0=h0, w_ih=w_ih, b_ih=b_ih, w_hh=w_hh, b_hh=b_hh,
            w_lin=w_lin, b_lin=b_lin,
        )
    )
    return out
